# revision 33
# baseline (speedup 1.0000x reference)
"""Axial attention block (B=8, C=512, H=W=128, 8 heads) on 8 Trainium2 cores.

Q/K/V projections in fp8(e4m3) DoubleRow mode (HW-microbenched 2.17x over
bf16 at N=512); weights pre-scaled by WS=16 (undone via the exp scale and
Wo); scores/AV/O-projection stay bf16.  Otherwise identical to the
HW-validated bf16 baseline structure.

Sharding: data-parallel over batch — one batch element per NeuronCore. Each
core runs both axial passes on its (C, H, W) slice and produces the full
residual sum xs + oh + ow.

Pass structure (all DMA contiguous-run; no strided DRAM access):
  - Pass 1 (HEIGHT attention, sequences along h, one per w): reads xtbf
    (C,W,H) chunks, computes oh tiles in (c, w-chunk, h) layout and writes
    them to a block-tiled DRAM scratch ohT2[hb][c, w, hi] (h = hb*8 + hi).
    The SBUF stage tile is laid out (hb, w, hi) so both DMA sides have
    >=32B/512B contiguous runs.
  - Pass 2 (WIDTH attention, sequences along w, one per h): h-chunk hb reads
    xbf + xs(f32) chunks and the matching ohT2[hb] block (contiguous), folds
    oh into the f32 residual once per chunk (GpSimd), then out = ow + resid
    per group (VectorE) and writes natural-layout output.

Matmul inputs are pre-cast to bf16 on the host (xbf natural, xtbf h<->w
swapped); weights pre-transposed to (c_in, c_out) bf16.

Per-sequence attention (S=128, dh=64): scoresT = K^T.T @ Q^T per head in
(s_k, s_q) layout, parity-split over two PSUM banks (concurrent row-group
matmuls must not share a bank); exp on ScalarE (max-subtraction skipped —
scaled scores are bounded ~7); denominators via ones-matmul on TensorE
(replicated rows); reciprocal_approx_fast + normalize on VectorE; AV packs
all 8 heads into one PSUM bank in (c, s_q) layout; O-projection batched
over 4 sequences.
"""
import os
import numpy as np
import ml_dtypes

P = 128          # partitions
C = 512          # channels
S = 128          # sequence length (H and W)
NCB = C // P     # channel blocks
NH = 8           # heads
DH = C // NH     # head dim
G = 4            # sequences per projection group
HC1 = 16         # w-chunk, height pass
HC2 = 8          # h-chunk, width pass (= hi block size of ohT2)
HB = S // HC2    # number of h blocks
NCORES = 8

_BF16 = ml_dtypes.bfloat16
_E4M3 = ml_dtypes.float8_e4m3
WS = 16.0

# schedule-tuning knobs (env-overridable for experiments)
PROJ_BUFS = int(os.environ.get("K_PROJ_BUFS", "2"))
ATTN_BUFS = int(os.environ.get("K_ATTN_BUFS", "2"))
ET_BUFS = int(os.environ.get("K_ET_BUFS", "2"))
QK_BUFS = int(os.environ.get("K_QK_BUFS", "2"))
VT_BUFS = int(os.environ.get("K_VT_BUFS", "2"))
OT_BUFS = int(os.environ.get("K_OT_BUFS", "2"))
RR_BUFS = int(os.environ.get("K_RR_BUFS", "2"))
PO_BUFS = int(os.environ.get("K_PO_BUFS", "2"))
VT_ON_ACT = int(os.environ.get("K_VT_ON_ACT", "0"))
STG1_ON_ACT = int(os.environ.get("K_STG1_ON_ACT", "0"))
QT_ENG = os.environ.get("K_QT_ENG", "act")
KT_ENG = os.environ.get("K_KT_ENG", "act")

_PROG = None  # cached compiled Bass program


def _build_program(reps=None):
    """Build the Bass program.  reps=None: normal external-I/O program.
    reps=R: timing variant — body wrapped in a hardware For_i loop run R
    times, inputs Internal (no host transfer)."""
    from contextlib import ExitStack
    import concourse.tile as tile
    from concourse import bacc, mybir

    f32 = mybir.dt.float32
    bf = mybir.dt.bfloat16
    f8 = mybir.dt.float8e4
    Exp = mybir.ActivationFunctionType.Exp
    DR = mybir.MatmulPerfMode.DoubleRow

    nc = bacc.Bacc("TRN2", target_bir_lowering=False, debug=False)

    timed = reps is not None
    kio = {} if timed else {"kind": "ExternalInput"}
    xf = nc.dram_tensor("xf", [C, S, S], f32, **kio).ap()
    xq8 = nc.dram_tensor("xq8", [P, NCB, S, S], f8, **kio).ap()   # (p, ci, w, h)
    xw8 = nc.dram_tensor("xw8", [P, NCB, S, S], f8, **kio).ap()   # (p, ci, h, w)
    w8names = ["wq_w", "wk_w", "wv_w", "wq_h", "wk_h", "wv_h"]
    w8t = {n: nc.dram_tensor(n, [P, NCB, C], f8, **kio).ap() for n in w8names}
    wonames = ["wo_w", "wo_h"]
    wot = {n: nc.dram_tensor(n, [C, C], bf, **kio).ap() for n in wonames}
    ohT2 = nc.dram_tensor("ohT2", [HB, C, S, HC2], f32).ap()
    out = nc.dram_tensor("out", [C, S, S], f32, kind="ExternalOutput").ap()

    with tile.TileContext(nc) as tc, ExitStack() as topctx:
        const = topctx.enter_context(tc.tile_pool(name="const", bufs=1))

        w_sb = {}
        for n in w8names:
            t = const.tile([P, NCB, C], f8, tag=f"w_{n}", name=f"w_{n}")
            nc.sync.dma_start(out=t, in_=w8t[n])
            w_sb[n] = t
        for n in wonames:
            tiles = []
            for ci in range(NCB):
                t = const.tile([P, C], bf, tag=f"w_{n}_{ci}", name=f"w_{n}_{ci}")
                nc.sync.dma_start(out=t, in_=wot[n][ci * P:(ci + 1) * P, :])
                tiles.append(t)
            w_sb[n] = tiles
        ones_sb = const.tile([P, P], bf, tag="ones", name="ones")
        nc.vector.memset(ones_sb, 1.0)

        def attn_group(x8_t, gsl, s0, wq, wk, wv, wo, pools):
            """One group of G sequences -> psum tiles of out-projection
            results, one (P, G*S) tile per c_out block."""
            qk_pool, vt_pool, ot_pool, et_pool, rr_pool, proj_ps, attn_ps, po_ps = pools

            qt_sb, kt_sb = [], []
            for wmat, dst_list, nm in ((wq, qt_sb, "qt"), (wk, kt_sb, "kt")):
                for co in range(NCB):
                    pp = proj_ps.tile([P, G * S], f32, tag="proj", name="pp")
                    for k2 in range(NCB // 2):
                        ks = slice(2 * k2, 2 * k2 + 2)
                        nc.tensor.matmul(
                            pp,
                            lhsT=wmat[:, ks, co * P:(co + 1) * P],
                            rhs=x8_t[:, ks, gsl, :],
                            start=(k2 == 0), stop=(k2 == NCB // 2 - 1),
                            perf_mode=DR)
                    sb_t = qk_pool.tile([P, G * S], bf, tag=f"{nm}{co}", name=f"{nm}{co}")
                    if (nm == "qt" and QT_ENG == "dve") or (nm == "kt" and KT_ENG == "dve"):
                        nc.vector.tensor_copy(sb_t, pp)
                    else:
                        nc.scalar.copy(sb_t, pp)
                    dst_list.append(sb_t)

            vt_sb = []
            for sq in range(G):
                pv = proj_ps.tile([P, C], f32, tag="proj", name="pv")
                for k2 in range(NCB // 2):
                    ks = slice(2 * k2, 2 * k2 + 2)
                    nc.tensor.matmul(
                        pv, lhsT=x8_t[:, ks, s0 + sq, :], rhs=wv[:, ks, :],
                        start=(k2 == 0), stop=(k2 == NCB // 2 - 1),
                        perf_mode=DR)
                vt = vt_pool.tile([P, C], bf, tag=f"vt{sq}", name=f"vt{sq}")
                if VT_ON_ACT:
                    nc.scalar.copy(vt, pv)
                else:
                    nc.vector.tensor_copy(vt, pv)
                vt_sb.append(vt)

            ot_full = ot_pool.tile([P, NCB, G * S], bf, tag="ot", name="ot")
            for sq in range(G):
                ssl = slice(sq * S, (sq + 1) * S)
                # scoresT: head h -> col h//2*128 of half (h%2); the two
                # 512-col halves are separate PSUM banks, so even (row-group
                # 0-63) and odd (64-127) head matmuls never share a bank
                st2 = attn_ps.tile([P, 1024], f32, tag="attn", name="st2")
                for h in range(NH):
                    par, cb = h % 2, h // 2
                    rows = slice(par * DH, (par + 1) * DH)
                    nc.tensor.matmul(
                        st2[:, par * 512 + cb * S:par * 512 + (cb + 1) * S],
                        lhsT=kt_sb[h // 2][rows, ssl],
                        rhs=qt_sb[h // 2][rows, ssl],
                        start=True, stop=True)
                et = et_pool.tile([P, 1024], bf, tag="et", name="et")
                nc.scalar.activation(out=et, in_=st2, func=Exp, scale=DH ** -0.5 / (WS * WS))
                r2 = attn_ps.tile([P, 1024], f32, tag="attn", name="r2")
                nc.tensor.matmul(r2[:, 0:512], lhsT=ones_sb, rhs=et[:, 0:512],
                                 start=True, stop=True)
                nc.tensor.matmul(r2[:, 512:1024], lhsT=ones_sb, rhs=et[:, 512:1024],
                                 start=True, stop=True)
                rr = rr_pool.tile([P, 1024], f32, tag="rr", name="rr")
                nc.vector.reciprocal_approx_fast(out=rr, in_=r2)
                # AV on unnormalized exp; softmax denominators commute past
                # the matmul (pure column scaling), so recip runs on VectorE
                # in parallel with AV on TensorE and the normalize fuses into
                # the psum->sbuf evacuation below.
                po = po_ps.tile([P, 512], f32, tag="po", name="po")
                for h in range(NH):
                    par, cb = h % 2, h // 2
                    nc.tensor.matmul(
                        po[par * DH:(par + 1) * DH, cb * S:(cb + 1) * S],
                        lhsT=vt_sb[sq][:, h * DH:(h + 1) * DH],
                        rhs=et[:, par * 512 + cb * S:par * 512 + (cb + 1) * S],
                        start=True, stop=True)
                # row-half r of po holds heads with parity r; its per-element
                # normalizer is exactly rr[:, r*512:] (rows replicated)
                nc.vector.tensor_mul(
                    ot_full[0:DH, :, ssl],
                    po[0:DH, :].rearrange("p (c s) -> p c s", c=NCB),
                    rr[0:DH, 0:512].rearrange("p (c s) -> p c s", c=NCB))
                nc.vector.tensor_mul(
                    ot_full[DH:P, :, ssl],
                    po[DH:P, :].rearrange("p (c s) -> p c s", c=NCB),
                    rr[DH:P, 512:1024].rearrange("p (c s) -> p c s", c=NCB))
            # O-projection outputs go through the attn pool's 2-bank tiles
            # (pairs of c_out blocks in the two bank halves) so the proj pool
            # frees up for the next group's Q/K/V immediately
            pods = []
            for cop in range(NCB // 2):
                pp2 = attn_ps.tile([P, 1024], f32, tag="attn", name="pp2")
                for half in range(2):
                    co = cop * 2 + half
                    dst = pp2[:, half * 512:(half + 1) * 512]
                    for ci in range(NCB):
                        nc.tensor.matmul(
                            dst,
                            lhsT=wo[ci][:, co * P:(co + 1) * P],
                            rhs=ot_full[:, ci, :],
                            start=(ci == 0), stop=(ci == NCB - 1))
                    pods.append(dst)
            return pods

        def height_pass():
            """Pass 1: height attention (seq along h, one per w).  Writes oh
            to the blocked scratch ohT2[hb][c, w, hi]."""
            wq, wk, wv, wo = (w_sb["wq_h"], w_sb["wk_h"], w_sb["wv_h"], w_sb["wo_h"])
            with ExitStack() as ctx:
                src_pool = ctx.enter_context(tc.tile_pool(name="src1", bufs=2))
                stage_pool = ctx.enter_context(tc.tile_pool(name="stg1", bufs=2))
                qk_pool = ctx.enter_context(tc.tile_pool(name="qk1", bufs=QK_BUFS))
                vt_pool = ctx.enter_context(tc.tile_pool(name="vt1", bufs=VT_BUFS))
                ot_pool = ctx.enter_context(tc.tile_pool(name="ot1", bufs=OT_BUFS))
                et_pool = ctx.enter_context(tc.tile_pool(name="et1", bufs=ET_BUFS))
                rr_pool = ctx.enter_context(tc.tile_pool(name="rr1", bufs=RR_BUFS))
                proj_ps = ctx.enter_context(tc.tile_pool(name="pps1", bufs=PROJ_BUFS, space="PSUM"))
                attn_ps = ctx.enter_context(tc.tile_pool(name="aps1", bufs=ATTN_BUFS, space="PSUM"))
                po_ps = ctx.enter_context(tc.tile_pool(name="pops1", bufs=PO_BUFS, space="PSUM"))
                pools = (qk_pool, vt_pool, ot_pool, et_pool, rr_pool, proj_ps, attn_ps, po_ps)

                for chunk in range(S // HC1):
                    q0 = chunk * HC1
                    src_t = src_pool.tile([P, NCB, HC1, S], f8, tag="src", name="src")
                    nc.sync.dma_start(out=src_t, in_=xq8[:, :, q0:q0 + HC1, :])
                    stage_t = []
                    for cb in range(NCB):
                        # stage layout (hb, w, hi): contiguous runs on both
                        # DMA sides of the blocked write
                        st = stage_pool.tile([P, HB, HC1, HC2], f32, tag=f"stg{cb}", name=f"stg{cb}")
                        stage_t.append(st)
                    for g in range(HC1 // G):
                        s0 = g * G
                        gsl = slice(s0, s0 + G)
                        pods = attn_group(src_t, gsl, s0, wq, wk, wv, wo, pools)
                        for co in range(NCB):
                            # pods: (p, 4 w-seq, 128 h) -> stage (hb, w in gsl, hi)
                            stage_op = nc.scalar.copy if STG1_ON_ACT else nc.vector.tensor_copy
                            stage_op(
                                stage_t[co][:, :, gsl, :].rearrange("p b q i -> p q b i"),
                                pods[co].rearrange("p (q b i) -> p q b i", q=G, b=HB))
                    for cb in range(NCB):
                        cs = slice(cb * P, (cb + 1) * P)
                        nc.sync.dma_start(
                            out=ohT2[:, cs, q0:q0 + HC1, :].rearrange("b c w i -> c b w i"),
                            in_=stage_t[cb])

        def width_pass():
            """Pass 2: width attention (seq along w, one per h).  h-chunk =
            hb block; out = xs + oh + ow in natural layout."""
            wq, wk, wv, wo = (w_sb["wq_w"], w_sb["wk_w"], w_sb["wv_w"], w_sb["wo_w"])
            with ExitStack() as ctx:
                src_pool = ctx.enter_context(tc.tile_pool(name="src2", bufs=2))
                resid_pool = ctx.enter_context(tc.tile_pool(name="res2", bufs=2))
                oh_pool = ctx.enter_context(tc.tile_pool(name="oh2", bufs=2))
                stage_pool = ctx.enter_context(tc.tile_pool(name="stg2", bufs=2))
                qk_pool = ctx.enter_context(tc.tile_pool(name="qk2", bufs=QK_BUFS))
                vt_pool = ctx.enter_context(tc.tile_pool(name="vt2", bufs=VT_BUFS))
                ot_pool = ctx.enter_context(tc.tile_pool(name="ot2", bufs=OT_BUFS))
                et_pool = ctx.enter_context(tc.tile_pool(name="et2", bufs=ET_BUFS))
                rr_pool = ctx.enter_context(tc.tile_pool(name="rr2", bufs=RR_BUFS))
                proj_ps = ctx.enter_context(tc.tile_pool(name="pps2", bufs=PROJ_BUFS, space="PSUM"))
                attn_ps = ctx.enter_context(tc.tile_pool(name="aps2", bufs=ATTN_BUFS, space="PSUM"))
                po_ps = ctx.enter_context(tc.tile_pool(name="pops2", bufs=PO_BUFS, space="PSUM"))
                pools = (qk_pool, vt_pool, ot_pool, et_pool, rr_pool, proj_ps, attn_ps, po_ps)

                for hb in range(HB):
                    q0 = hb * HC2
                    src_t = src_pool.tile([P, NCB, HC2, S], f8, tag="src", name="src")
                    nc.sync.dma_start(out=src_t, in_=xw8[:, :, q0:q0 + HC2, :])
                    resid_t, stage_t = [], []
                    for cb in range(NCB):
                        cs = slice(cb * P, (cb + 1) * P)
                        rt = resid_pool.tile([P, HC2, S], f32, tag=f"res{cb}", name=f"res{cb}")
                        nc.sync.dma_start(out=rt, in_=xf[cs, q0:q0 + HC2, :])
                        resid_t.append(rt)
                        oht = oh_pool.tile([P, S, HC2], f32, tag=f"oh{cb}", name=f"oh{cb}")
                        nc.sync.dma_start(out=oht, in_=ohT2[hb, cs, :, :])
                        # fold oh into the residual once per chunk
                        nc.gpsimd.tensor_tensor(
                            out=rt, in0=rt,
                            in1=oht.rearrange("p w i -> p i w"),
                            op=mybir.AluOpType.add)
                        st = stage_pool.tile([P, HC2, S], f32, tag=f"stg{cb}", name=f"stg{cb}")
                        stage_t.append(st)
                    for g in range(HC2 // G):
                        s0 = g * G
                        gsl = slice(s0, s0 + G)
                        pods = attn_group(src_t, gsl, s0, wq, wk, wv, wo, pools)
                        for co in range(NCB):
                            nc.vector.tensor_add(
                                stage_t[co][:, gsl, :],
                                pods[co].rearrange("p (q s) -> p q s", q=G),
                                resid_t[co][:, gsl, :])
                    for cb in range(NCB):
                        cs = slice(cb * P, (cb + 1) * P)
                        nc.sync.dma_start(out=out[cs, q0:q0 + HC2, :], in_=stage_t[cb])

        if timed:
            with tc.For_i(0, reps):
                height_pass()
                width_pass()
        else:
            height_pass()
            width_pass()

    nc.compile()
    return nc


def _get_program():
    global _PROG
    if _PROG is None:
        _PROG = _build_program()
    return _PROG


def kernel(xs, Wq_h, Wk_h, Wv_h, Wo_h, Wq_w, Wk_w, Wv_w, Wo_w):
    from concourse.bass_utils import run_bass_kernel_spmd

    nc = _get_program()

    w8map = {
        "wq_w": Wq_w, "wk_w": Wk_w, "wv_w": Wv_w,
        "wq_h": Wq_h, "wk_h": Wk_h, "wv_h": Wv_h,
    }
    w8_np = {}
    for n, w in w8map.items():
        wt = np.asarray(w, dtype=np.float32).T * WS          # (c_in, c_out)
        w8 = np.clip(wt, -240, 240).astype(_E4M3)
        w8_np[n] = np.ascontiguousarray(
            w8.reshape(NCB, P, C).transpose(1, 0, 2))        # (p, ci, co)
    wo_np = {
        "wo_w": np.ascontiguousarray(np.asarray(Wo_w, dtype=np.float32).T / WS).astype(_BF16),
        "wo_h": np.ascontiguousarray(np.asarray(Wo_h, dtype=np.float32).T / WS).astype(_BF16),
    }

    xs = np.asarray(xs, dtype=np.float32)
    in_maps = []
    for b in range(NCORES):
        xb = np.ascontiguousarray(xs[b])                        # (C, H, W) f32
        x8n = np.clip(xb, -240, 240).astype(_E4M3)
        xw8 = np.ascontiguousarray(
            x8n.reshape(NCB, P, S, S).transpose(1, 0, 2, 3))    # (p, ci, h, w)
        x8s = np.ascontiguousarray(np.swapaxes(x8n, 1, 2))      # (C, W, H)
        xq8 = np.ascontiguousarray(
            x8s.reshape(NCB, P, S, S).transpose(1, 0, 2, 3))    # (p, ci, w, h)
        in_maps.append({"xf": xb, "xq8": xq8, "xw8": xw8, **w8_np, **wo_np})

    res = run_bass_kernel_spmd(nc, in_maps, core_ids=list(range(NCORES)))
    return np.stack([res.results[b]["out"] for b in range(NCORES)], axis=0)



# revision 35
# speedup vs baseline: 1.4392x; 1.4392x over previous
"""Axial attention block (B=8, C=512, H=W=128, 8 heads) on 8 Trainium2 cores.

Q/K/V projections in fp8(e4m3) DoubleRow mode (HW-microbenched 2.17x over
bf16 at N=512); weights pre-scaled by WS=16 (undone via the exp scale and
Wo); scores/AV/O-projection stay bf16.  Otherwise identical to the
HW-validated bf16 baseline structure.

Sharding: data-parallel over batch — one batch element per NeuronCore. Each
core runs both axial passes on its (C, H, W) slice and produces the full
residual sum xs + oh + ow.

Pass structure (all DMA contiguous-run; no strided DRAM access):
  - Pass 1 (HEIGHT attention, sequences along h, one per w): reads xtbf
    (C,W,H) chunks, computes oh tiles in (c, w-chunk, h) layout and writes
    them to a block-tiled DRAM scratch ohT2[hb][c, w, hi] (h = hb*8 + hi).
    The SBUF stage tile is laid out (hb, w, hi) so both DMA sides have
    >=32B/512B contiguous runs.
  - Pass 2 (WIDTH attention, sequences along w, one per h): h-chunk hb reads
    xbf + xs(f32) chunks and the matching ohT2[hb] block (contiguous), folds
    oh into the f32 residual once per chunk (GpSimd), then out = ow + resid
    per group (VectorE) and writes natural-layout output.

Matmul inputs are pre-cast to bf16 on the host (xbf natural, xtbf h<->w
swapped); weights pre-transposed to (c_in, c_out) bf16.

Per-sequence attention (S=128, dh=64): scoresT = K^T.T @ Q^T per head in
(s_k, s_q) layout, parity-split over two PSUM banks (concurrent row-group
matmuls must not share a bank); exp on ScalarE (max-subtraction skipped —
scaled scores are bounded ~7); denominators via ones-matmul on TensorE
(replicated rows); reciprocal_approx_fast + normalize on VectorE; AV packs
all 8 heads into one PSUM bank in (c, s_q) layout; O-projection batched
over 4 sequences.
"""
import os
import numpy as np
import ml_dtypes

P = 128          # partitions
C = 512          # channels
S = 128          # sequence length (H and W)
NCB = C // P     # channel blocks
NH = 8           # heads
DH = C // NH     # head dim
G = 4            # sequences per projection group
HC1 = 16         # w-chunk, height pass
HC2 = 8          # h-chunk, width pass (= hi block size of ohT2)
HB = S // HC2    # number of h blocks
NCORES = 8

_BF16 = ml_dtypes.bfloat16
_E4M3 = ml_dtypes.float8_e4m3
WS = 16.0

# schedule-tuning knobs (env-overridable for experiments)
PROJ_BUFS = int(os.environ.get("K_PROJ_BUFS", "2"))
ATTN_BUFS = int(os.environ.get("K_ATTN_BUFS", "2"))
ET_BUFS = int(os.environ.get("K_ET_BUFS", "2"))
QK_BUFS = int(os.environ.get("K_QK_BUFS", "2"))
VT_BUFS = int(os.environ.get("K_VT_BUFS", "2"))
OT_BUFS = int(os.environ.get("K_OT_BUFS", "2"))
RR_BUFS = int(os.environ.get("K_RR_BUFS", "2"))
PO_BUFS = int(os.environ.get("K_PO_BUFS", "2"))
VT_ON_ACT = int(os.environ.get("K_VT_ON_ACT", "0"))
STG1_ON_ACT = int(os.environ.get("K_STG1_ON_ACT", "0"))
QT_ENG = os.environ.get("K_QT_ENG", "act")
KT_ENG = os.environ.get("K_KT_ENG", "act")

_PROG = None  # cached compiled Bass program


def _build_program(reps=None):
    """Build the Bass program.  reps=None: normal external-I/O program.
    reps=R: timing variant — body wrapped in a hardware For_i loop run R
    times, inputs Internal (no host transfer)."""
    from contextlib import ExitStack
    import concourse.tile as tile
    from concourse import bacc, mybir

    f32 = mybir.dt.float32
    bf = mybir.dt.bfloat16
    f8 = mybir.dt.float8e4
    Exp = mybir.ActivationFunctionType.Exp
    DR = mybir.MatmulPerfMode.DoubleRow

    nc = bacc.Bacc("TRN2", target_bir_lowering=False, debug=False)

    timed = reps is not None
    kio = {} if timed else {"kind": "ExternalInput"}
    xf = nc.dram_tensor("xf", [C, S, S], f32, **kio).ap()
    xq8 = nc.dram_tensor("xq8", [P, NCB, S, S], f8, **kio).ap()   # (p, ci, w, h)
    xw8 = nc.dram_tensor("xw8", [P, NCB, S, S], f8, **kio).ap()   # (p, ci, h, w)
    w8names = ["wq_w", "wk_w", "wv_w", "wq_h", "wk_h", "wv_h"]
    w8t = {n: nc.dram_tensor(n, [P, NCB, C], f8, **kio).ap() for n in w8names}
    wonames = ["wo_w", "wo_h"]
    wot = {n: nc.dram_tensor(n, [C, C], bf, **kio).ap() for n in wonames}
    ohT2 = nc.dram_tensor("ohT2", [HB, C, S, HC2], f32).ap()
    out = nc.dram_tensor("out", [C, S, S], f32, kind="ExternalOutput").ap()

    with tile.TileContext(nc) as tc, ExitStack() as topctx:
        const = topctx.enter_context(tc.tile_pool(name="const", bufs=1))

        w_sb = {}
        for n in w8names:
            t = const.tile([P, NCB, C], f8, tag=f"w_{n}", name=f"w_{n}")
            nc.sync.dma_start(out=t, in_=w8t[n])
            w_sb[n] = t
        for n in wonames:
            tiles = []
            for ci in range(NCB):
                t = const.tile([P, C], bf, tag=f"w_{n}_{ci}", name=f"w_{n}_{ci}")
                nc.sync.dma_start(out=t, in_=wot[n][ci * P:(ci + 1) * P, :])
                tiles.append(t)
            w_sb[n] = tiles
        ones_sb = const.tile([P, P], bf, tag="ones", name="ones")
        nc.vector.memset(ones_sb, 1.0)

        def attn_group(x8_t, gsl, s0, wq, wk, wv, wo, pools):
            """One group of G sequences -> psum tiles of out-projection
            results, one (P, G*S) tile per c_out block."""
            qk_pool, vt_pool, ot_pool, et_pool, rr_pool, proj_ps, attn_ps, po_ps = pools

            qt_sb, kt_sb = [], []
            for wmat, dst_list, nm in ((wq, qt_sb, "qt"), (wk, kt_sb, "kt")):
                for co in range(NCB):
                    pp = proj_ps.tile([P, G * S], f32, tag="proj", name="pp")
                    for k2 in range(NCB // 2):
                        ks = slice(2 * k2, 2 * k2 + 2)
                        nc.tensor.matmul(
                            pp,
                            lhsT=wmat[:, ks, co * P:(co + 1) * P],
                            rhs=x8_t[:, ks, gsl, :],
                            start=(k2 == 0), stop=(k2 == NCB // 2 - 1),
                            perf_mode=DR)
                    sb_t = qk_pool.tile([P, G * S], bf, tag=f"{nm}{co}", name=f"{nm}{co}")
                    if (nm == "qt" and QT_ENG == "dve") or (nm == "kt" and KT_ENG == "dve"):
                        nc.vector.tensor_copy(sb_t, pp)
                    else:
                        nc.scalar.copy(sb_t, pp)
                    dst_list.append(sb_t)

            vt_sb = []
            for sq in range(G):
                pv = proj_ps.tile([P, C], f32, tag="proj", name="pv")
                for k2 in range(NCB // 2):
                    ks = slice(2 * k2, 2 * k2 + 2)
                    nc.tensor.matmul(
                        pv, lhsT=x8_t[:, ks, s0 + sq, :], rhs=wv[:, ks, :],
                        start=(k2 == 0), stop=(k2 == NCB // 2 - 1),
                        perf_mode=DR)
                vt = vt_pool.tile([P, C], bf, tag=f"vt{sq}", name=f"vt{sq}")
                if VT_ON_ACT:
                    nc.scalar.copy(vt, pv)
                else:
                    nc.vector.tensor_copy(vt, pv)
                vt_sb.append(vt)

            ot_full = ot_pool.tile([P, NCB, G * S], bf, tag="ot", name="ot")
            for sq in range(G):
                ssl = slice(sq * S, (sq + 1) * S)
                # scoresT: head h -> col h//2*128 of half (h%2); the two
                # 512-col halves are separate PSUM banks, so even (row-group
                # 0-63) and odd (64-127) head matmuls never share a bank
                st2 = attn_ps.tile([P, 1024], f32, tag="attn", name="st2")
                for h in range(NH):
                    par, cb = h % 2, h // 2
                    rows = slice(par * DH, (par + 1) * DH)
                    nc.tensor.matmul(
                        st2[:, par * 512 + cb * S:par * 512 + (cb + 1) * S],
                        lhsT=kt_sb[h // 2][rows, ssl],
                        rhs=qt_sb[h // 2][rows, ssl],
                        start=True, stop=True)
                et = et_pool.tile([P, 1024], bf, tag="et", name="et")
                nc.scalar.activation(out=et, in_=st2, func=Exp, scale=DH ** -0.5 / (WS * WS))
                r2 = attn_ps.tile([P, 1024], f32, tag="attn", name="r2")
                nc.tensor.matmul(r2[:, 0:512], lhsT=ones_sb, rhs=et[:, 0:512],
                                 start=True, stop=True)
                nc.tensor.matmul(r2[:, 512:1024], lhsT=ones_sb, rhs=et[:, 512:1024],
                                 start=True, stop=True)
                rr = rr_pool.tile([P, 1024], f32, tag="rr", name="rr")
                nc.vector.reciprocal_approx_fast(out=rr, in_=r2)
                # AV on unnormalized exp; softmax denominators commute past
                # the matmul (pure column scaling), so recip runs on VectorE
                # in parallel with AV on TensorE and the normalize fuses into
                # the psum->sbuf evacuation below.
                po = po_ps.tile([P, 512], f32, tag="po", name="po")
                for h in range(NH):
                    par, cb = h % 2, h // 2
                    nc.tensor.matmul(
                        po[par * DH:(par + 1) * DH, cb * S:(cb + 1) * S],
                        lhsT=vt_sb[sq][:, h * DH:(h + 1) * DH],
                        rhs=et[:, par * 512 + cb * S:par * 512 + (cb + 1) * S],
                        start=True, stop=True)
                # row-half r of po holds heads with parity r; its per-element
                # normalizer is exactly rr[:, r*512:] (rows replicated)
                nc.vector.tensor_mul(
                    ot_full[0:DH, :, ssl],
                    po[0:DH, :].rearrange("p (c s) -> p c s", c=NCB),
                    rr[0:DH, 0:512].rearrange("p (c s) -> p c s", c=NCB))
                nc.vector.tensor_mul(
                    ot_full[DH:P, :, ssl],
                    po[DH:P, :].rearrange("p (c s) -> p c s", c=NCB),
                    rr[DH:P, 512:1024].rearrange("p (c s) -> p c s", c=NCB))
            # O-projection outputs go through the attn pool's 2-bank tiles
            # (pairs of c_out blocks in the two bank halves) so the proj pool
            # frees up for the next group's Q/K/V immediately
            pods = []
            for cop in range(NCB // 2):
                pp2 = attn_ps.tile([P, 1024], f32, tag="attn", name="pp2")
                for half in range(2):
                    co = cop * 2 + half
                    dst = pp2[:, half * 512:(half + 1) * 512]
                    for ci in range(NCB):
                        nc.tensor.matmul(
                            dst,
                            lhsT=wo[ci][:, co * P:(co + 1) * P],
                            rhs=ot_full[:, ci, :],
                            start=(ci == 0), stop=(ci == NCB - 1))
                    pods.append(dst)
            return pods

        def height_pass():
            """Pass 1: height attention (seq along h, one per w).  Writes oh
            to the blocked scratch ohT2[hb][c, w, hi]."""
            wq, wk, wv, wo = (w_sb["wq_h"], w_sb["wk_h"], w_sb["wv_h"], w_sb["wo_h"])
            with ExitStack() as ctx:
                src_pool = ctx.enter_context(tc.tile_pool(name="src1", bufs=2))
                stage_pool = ctx.enter_context(tc.tile_pool(name="stg1", bufs=2))
                qk_pool = ctx.enter_context(tc.tile_pool(name="qk1", bufs=QK_BUFS))
                vt_pool = ctx.enter_context(tc.tile_pool(name="vt1", bufs=VT_BUFS))
                ot_pool = ctx.enter_context(tc.tile_pool(name="ot1", bufs=OT_BUFS))
                et_pool = ctx.enter_context(tc.tile_pool(name="et1", bufs=ET_BUFS))
                rr_pool = ctx.enter_context(tc.tile_pool(name="rr1", bufs=RR_BUFS))
                proj_ps = ctx.enter_context(tc.tile_pool(name="pps1", bufs=PROJ_BUFS, space="PSUM"))
                attn_ps = ctx.enter_context(tc.tile_pool(name="aps1", bufs=ATTN_BUFS, space="PSUM"))
                po_ps = ctx.enter_context(tc.tile_pool(name="pops1", bufs=PO_BUFS, space="PSUM"))
                pools = (qk_pool, vt_pool, ot_pool, et_pool, rr_pool, proj_ps, attn_ps, po_ps)

                for chunk in range(S // HC1):
                    q0 = chunk * HC1
                    src_t = src_pool.tile([P, NCB, HC1, S], f8, tag="src", name="src")
                    nc.sync.dma_start(out=src_t, in_=xq8[:, :, q0:q0 + HC1, :])
                    stage_t = []
                    for cb in range(NCB):
                        # stage layout (hb, w, hi): contiguous runs on both
                        # DMA sides of the blocked write
                        st = stage_pool.tile([P, HB, HC1, HC2], f32, tag=f"stg{cb}", name=f"stg{cb}")
                        stage_t.append(st)
                    for g in range(HC1 // G):
                        s0 = g * G
                        gsl = slice(s0, s0 + G)
                        pods = attn_group(src_t, gsl, s0, wq, wk, wv, wo, pools)
                        for co in range(NCB):
                            # pods: (p, 4 w-seq, 128 h) -> stage (hb, w in gsl, hi)
                            stage_op = nc.scalar.copy if STG1_ON_ACT else nc.vector.tensor_copy
                            stage_op(
                                stage_t[co][:, :, gsl, :].rearrange("p b q i -> p q b i"),
                                pods[co].rearrange("p (q b i) -> p q b i", q=G, b=HB))
                    for cb in range(NCB):
                        cs = slice(cb * P, (cb + 1) * P)
                        nc.sync.dma_start(
                            out=ohT2[:, cs, q0:q0 + HC1, :].rearrange("b c w i -> c b w i"),
                            in_=stage_t[cb])

        def width_pass():
            """Pass 2: width attention (seq along w, one per h).  h-chunk =
            hb block; out = xs + oh + ow in natural layout."""
            wq, wk, wv, wo = (w_sb["wq_w"], w_sb["wk_w"], w_sb["wv_w"], w_sb["wo_w"])
            with ExitStack() as ctx:
                src_pool = ctx.enter_context(tc.tile_pool(name="src2", bufs=2))
                resid_pool = ctx.enter_context(tc.tile_pool(name="res2", bufs=2))
                oh_pool = ctx.enter_context(tc.tile_pool(name="oh2", bufs=2))
                stage_pool = ctx.enter_context(tc.tile_pool(name="stg2", bufs=2))
                qk_pool = ctx.enter_context(tc.tile_pool(name="qk2", bufs=QK_BUFS))
                vt_pool = ctx.enter_context(tc.tile_pool(name="vt2", bufs=VT_BUFS))
                ot_pool = ctx.enter_context(tc.tile_pool(name="ot2", bufs=OT_BUFS))
                et_pool = ctx.enter_context(tc.tile_pool(name="et2", bufs=ET_BUFS))
                rr_pool = ctx.enter_context(tc.tile_pool(name="rr2", bufs=RR_BUFS))
                proj_ps = ctx.enter_context(tc.tile_pool(name="pps2", bufs=PROJ_BUFS, space="PSUM"))
                attn_ps = ctx.enter_context(tc.tile_pool(name="aps2", bufs=ATTN_BUFS, space="PSUM"))
                po_ps = ctx.enter_context(tc.tile_pool(name="pops2", bufs=PO_BUFS, space="PSUM"))
                pools = (qk_pool, vt_pool, ot_pool, et_pool, rr_pool, proj_ps, attn_ps, po_ps)

                for hb in range(HB):
                    q0 = hb * HC2
                    src_t = src_pool.tile([P, NCB, HC2, S], f8, tag="src", name="src")
                    nc.sync.dma_start(out=src_t, in_=xw8[:, :, q0:q0 + HC2, :])
                    resid_t, stage_t = [], []
                    for cb in range(NCB):
                        cs = slice(cb * P, (cb + 1) * P)
                        rt = resid_pool.tile([P, HC2, S], f32, tag=f"res{cb}", name=f"res{cb}")
                        nc.sync.dma_start(out=rt, in_=xf[cs, q0:q0 + HC2, :])
                        resid_t.append(rt)
                        oht = oh_pool.tile([P, S, HC2], f32, tag=f"oh{cb}", name=f"oh{cb}")
                        nc.sync.dma_start(out=oht, in_=ohT2[hb, cs, :, :])
                        # fold oh into the residual once per chunk
                        nc.gpsimd.tensor_tensor(
                            out=rt, in0=rt,
                            in1=oht.rearrange("p w i -> p i w"),
                            op=mybir.AluOpType.add)
                        st = stage_pool.tile([P, HC2, S], f32, tag=f"stg{cb}", name=f"stg{cb}")
                        stage_t.append(st)
                    for g in range(HC2 // G):
                        s0 = g * G
                        gsl = slice(s0, s0 + G)
                        pods = attn_group(src_t, gsl, s0, wq, wk, wv, wo, pools)
                        for co in range(NCB):
                            nc.vector.tensor_add(
                                stage_t[co][:, gsl, :],
                                pods[co].rearrange("p (q s) -> p q s", q=G),
                                resid_t[co][:, gsl, :])
                    for cb in range(NCB):
                        cs = slice(cb * P, (cb + 1) * P)
                        nc.sync.dma_start(out=out[cs, q0:q0 + HC2, :], in_=stage_t[cb])

        if timed:
            with tc.For_i(0, reps):
                height_pass()
                width_pass()
        else:
            height_pass()
            width_pass()

    nc.compile()
    return nc


def _get_program():
    global _PROG
    if _PROG is None:
        _PROG = _build_program()
    return _PROG


def kernel(xs, Wq_h, Wk_h, Wv_h, Wo_h, Wq_w, Wk_w, Wv_w, Wo_w):
    from concourse.bass_utils import run_bass_kernel_spmd

    nc = _get_program()

    w8map = {
        "wq_w": Wq_w, "wk_w": Wk_w, "wv_w": Wv_w,
        "wq_h": Wq_h, "wk_h": Wk_h, "wv_h": Wv_h,
    }
    w8_np = {}
    for n, w in w8map.items():
        wt = np.asarray(w, dtype=np.float32).T * WS          # (c_in, c_out)
        w8 = np.clip(wt, -240, 240).astype(_E4M3)
        w8_np[n] = np.ascontiguousarray(
            w8.reshape(NCB, P, C).transpose(1, 0, 2))        # (p, ci, co)
    wo_np = {
        "wo_w": np.ascontiguousarray(np.asarray(Wo_w, dtype=np.float32).T / WS).astype(_BF16),
        "wo_h": np.ascontiguousarray(np.asarray(Wo_h, dtype=np.float32).T / WS).astype(_BF16),
    }

    xs = np.asarray(xs, dtype=np.float32)
    in_maps = []
    for b in range(NCORES):
        xb = np.ascontiguousarray(xs[b])                        # (C, H, W) f32
        x8n = np.clip(xb, -240, 240).astype(_E4M3)
        xw8 = np.ascontiguousarray(
            x8n.reshape(NCB, P, S, S).transpose(1, 0, 2, 3))    # (p, ci, h, w)
        x8s = np.ascontiguousarray(np.swapaxes(x8n, 1, 2))      # (C, W, H)
        xq8 = np.ascontiguousarray(
            x8s.reshape(NCB, P, S, S).transpose(1, 0, 2, 3))    # (p, ci, w, h)
        in_maps.append({"xf": xb, "xq8": xq8, "xw8": xw8, **w8_np, **wo_np})

    res = run_bass_kernel_spmd(nc, in_maps, core_ids=list(range(NCORES)))
    return np.stack([res.results[b]["out"] for b in range(NCORES)], axis=0)



# revision 36
# speedup vs baseline: 1.8858x; 1.3103x over previous
"""Axial attention block (B=8, C=512, H=W=128, 8 heads) on 8 Trainium2 cores.

Q/K/V projections in fp8(e4m3) DoubleRow mode (HW-microbenched 2.17x over
bf16 at N=512); weights pre-scaled by WS=16 (undone via the exp scale and
Wo); scores/AV/O-projection stay bf16.  Otherwise identical to the
HW-validated bf16 baseline structure.

Sharding: data-parallel over batch — one batch element per NeuronCore. Each
core runs both axial passes on its (C, H, W) slice and produces the full
residual sum xs + oh + ow.

Pass structure (all DMA contiguous-run; no strided DRAM access):
  - Pass 1 (HEIGHT attention, sequences along h, one per w): reads xtbf
    (C,W,H) chunks, computes oh tiles in (c, w-chunk, h) layout and writes
    them to a block-tiled DRAM scratch ohT2[hb][c, w, hi] (h = hb*8 + hi).
    The SBUF stage tile is laid out (hb, w, hi) so both DMA sides have
    >=32B/512B contiguous runs.
  - Pass 2 (WIDTH attention, sequences along w, one per h): h-chunk hb reads
    xbf + xs(f32) chunks and the matching ohT2[hb] block (contiguous), folds
    oh into the f32 residual once per chunk (GpSimd), then out = ow + resid
    per group (VectorE) and writes natural-layout output.

Matmul inputs are pre-cast to bf16 on the host (xbf natural, xtbf h<->w
swapped); weights pre-transposed to (c_in, c_out) bf16.

Per-sequence attention (S=128, dh=64): scoresT = K^T.T @ Q^T per head in
(s_k, s_q) layout, parity-split over two PSUM banks (concurrent row-group
matmuls must not share a bank); exp on ScalarE (max-subtraction skipped —
scaled scores are bounded ~7); denominators via ones-matmul on TensorE
(replicated rows); reciprocal_approx_fast + normalize on VectorE; AV packs
all 8 heads into one PSUM bank in (c, s_q) layout; O-projection batched
over 4 sequences.
"""
import os
import numpy as np
import ml_dtypes

P = 128          # partitions
C = 512          # channels
S = 128          # sequence length (H and W)
NCB = C // P     # channel blocks
NH = 8           # heads
DH = C // NH     # head dim
G = 4            # sequences per projection group
HC1 = 16         # w-chunk, height pass
HC2 = 8          # h-chunk, width pass (= hi block size of ohT2)
HB = S // HC2    # number of h blocks
NCORES = 8

_BF16 = ml_dtypes.bfloat16
_E4M3 = ml_dtypes.float8_e4m3
WS = 16.0

# schedule-tuning knobs (env-overridable for experiments)
PROJ_BUFS = int(os.environ.get("K_PROJ_BUFS", "2"))
ATTN_BUFS = int(os.environ.get("K_ATTN_BUFS", "2"))
ET_BUFS = int(os.environ.get("K_ET_BUFS", "2"))
QK_BUFS = int(os.environ.get("K_QK_BUFS", "2"))
VT_BUFS = int(os.environ.get("K_VT_BUFS", "2"))
OT_BUFS = int(os.environ.get("K_OT_BUFS", "2"))
RR_BUFS = int(os.environ.get("K_RR_BUFS", "2"))
PO_BUFS = int(os.environ.get("K_PO_BUFS", "2"))
VT_ON_ACT = int(os.environ.get("K_VT_ON_ACT", "1"))
STG1_ON_ACT = int(os.environ.get("K_STG1_ON_ACT", "0"))
QT_ENG = os.environ.get("K_QT_ENG", "act")
KT_ENG = os.environ.get("K_KT_ENG", "act")

_PROG = None  # cached compiled Bass program


def _build_program(reps=None):
    """Build the Bass program.  reps=None: normal external-I/O program.
    reps=R: timing variant — body wrapped in a hardware For_i loop run R
    times, inputs Internal (no host transfer)."""
    from contextlib import ExitStack
    import concourse.tile as tile
    from concourse import bacc, mybir

    f32 = mybir.dt.float32
    bf = mybir.dt.bfloat16
    f8 = mybir.dt.float8e4
    Exp = mybir.ActivationFunctionType.Exp
    DR = mybir.MatmulPerfMode.DoubleRow

    nc = bacc.Bacc("TRN2", target_bir_lowering=False, debug=False)

    timed = reps is not None
    kio = {} if timed else {"kind": "ExternalInput"}
    xf = nc.dram_tensor("xf", [C, S, S], f32, **kio).ap()
    xq8 = nc.dram_tensor("xq8", [P, NCB, S, S], f8, **kio).ap()   # (p, ci, w, h)
    xw8 = nc.dram_tensor("xw8", [P, NCB, S, S], f8, **kio).ap()   # (p, ci, h, w)
    w8names = ["wq_w", "wk_w", "wv_w", "wq_h", "wk_h", "wv_h"]
    w8t = {n: nc.dram_tensor(n, [P, NCB, C], f8, **kio).ap() for n in w8names}
    wonames = ["wo_w", "wo_h"]
    wot = {n: nc.dram_tensor(n, [C, C], bf, **kio).ap() for n in wonames}
    ohT2 = nc.dram_tensor("ohT2", [HB, C, S, HC2], f32).ap()
    out = nc.dram_tensor("out", [C, S, S], f32, kind="ExternalOutput").ap()

    with tile.TileContext(nc) as tc, ExitStack() as topctx:
        const = topctx.enter_context(tc.tile_pool(name="const", bufs=1))

        w_sb = {}
        for n in w8names:
            t = const.tile([P, NCB, C], f8, tag=f"w_{n}", name=f"w_{n}")
            nc.sync.dma_start(out=t, in_=w8t[n])
            w_sb[n] = t
        for n in wonames:
            tiles = []
            for ci in range(NCB):
                t = const.tile([P, C], bf, tag=f"w_{n}_{ci}", name=f"w_{n}_{ci}")
                nc.sync.dma_start(out=t, in_=wot[n][ci * P:(ci + 1) * P, :])
                tiles.append(t)
            w_sb[n] = tiles
        ones_sb = const.tile([P, P], bf, tag="ones", name="ones")
        nc.vector.memset(ones_sb, 1.0)

        def attn_group(x8_t, gsl, s0, wq, wk, wv, wo, pools):
            """One group of G sequences -> psum tiles of out-projection
            results, one (P, G*S) tile per c_out block."""
            qk_pool, vt_pool, ot_pool, et_pool, rr_pool, proj_ps, attn_ps, po_ps = pools

            qt_sb, kt_sb = [], []
            for wmat, dst_list, nm in ((wq, qt_sb, "qt"), (wk, kt_sb, "kt")):
                for co in range(NCB):
                    pp = proj_ps.tile([P, G * S], f32, tag="proj", name="pp")
                    for k2 in range(NCB // 2):
                        ks = slice(2 * k2, 2 * k2 + 2)
                        nc.tensor.matmul(
                            pp,
                            lhsT=wmat[:, ks, co * P:(co + 1) * P],
                            rhs=x8_t[:, ks, gsl, :],
                            start=(k2 == 0), stop=(k2 == NCB // 2 - 1),
                            perf_mode=DR)
                    sb_t = qk_pool.tile([P, G * S], bf, tag=f"{nm}{co}", name=f"{nm}{co}")
                    if (nm == "qt" and QT_ENG == "dve") or (nm == "kt" and KT_ENG == "dve"):
                        nc.vector.tensor_copy(sb_t, pp)
                    else:
                        nc.scalar.copy(sb_t, pp)
                    dst_list.append(sb_t)

            vt_sb = []
            for sq in range(G):
                pv = proj_ps.tile([P, C], f32, tag="proj", name="pv")
                for k2 in range(NCB // 2):
                    ks = slice(2 * k2, 2 * k2 + 2)
                    nc.tensor.matmul(
                        pv, lhsT=x8_t[:, ks, s0 + sq, :], rhs=wv[:, ks, :],
                        start=(k2 == 0), stop=(k2 == NCB // 2 - 1),
                        perf_mode=DR)
                vt = vt_pool.tile([P, C], bf, tag=f"vt{sq}", name=f"vt{sq}")
                if VT_ON_ACT:
                    nc.scalar.copy(vt, pv)
                else:
                    nc.vector.tensor_copy(vt, pv)
                vt_sb.append(vt)

            ot_full = ot_pool.tile([P, NCB, G * S], bf, tag="ot", name="ot")
            for sq in range(G):
                ssl = slice(sq * S, (sq + 1) * S)
                # scoresT: head h -> col h//2*128 of half (h%2); the two
                # 512-col halves are separate PSUM banks, so even (row-group
                # 0-63) and odd (64-127) head matmuls never share a bank
                st2 = attn_ps.tile([P, 1024], f32, tag="attn", name="st2")
                for h in range(NH):
                    par, cb = h % 2, h // 2
                    rows = slice(par * DH, (par + 1) * DH)
                    nc.tensor.matmul(
                        st2[:, par * 512 + cb * S:par * 512 + (cb + 1) * S],
                        lhsT=kt_sb[h // 2][rows, ssl],
                        rhs=qt_sb[h // 2][rows, ssl],
                        start=True, stop=True)
                et = et_pool.tile([P, 1024], bf, tag="et", name="et")
                nc.scalar.activation(out=et, in_=st2, func=Exp, scale=DH ** -0.5 / (WS * WS))
                r2 = attn_ps.tile([P, 1024], f32, tag="attn", name="r2")
                nc.tensor.matmul(r2[:, 0:512], lhsT=ones_sb, rhs=et[:, 0:512],
                                 start=True, stop=True)
                nc.tensor.matmul(r2[:, 512:1024], lhsT=ones_sb, rhs=et[:, 512:1024],
                                 start=True, stop=True)
                rr = rr_pool.tile([P, 1024], f32, tag="rr", name="rr")
                nc.vector.reciprocal_approx_fast(out=rr, in_=r2)
                # AV on unnormalized exp; softmax denominators commute past
                # the matmul (pure column scaling), so recip runs on VectorE
                # in parallel with AV on TensorE and the normalize fuses into
                # the psum->sbuf evacuation below.
                po = po_ps.tile([P, 512], f32, tag="po", name="po")
                for h in range(NH):
                    par, cb = h % 2, h // 2
                    nc.tensor.matmul(
                        po[par * DH:(par + 1) * DH, cb * S:(cb + 1) * S],
                        lhsT=vt_sb[sq][:, h * DH:(h + 1) * DH],
                        rhs=et[:, par * 512 + cb * S:par * 512 + (cb + 1) * S],
                        start=True, stop=True)
                # row-half r of po holds heads with parity r; its per-element
                # normalizer is exactly rr[:, r*512:] (rows replicated)
                nc.vector.tensor_mul(
                    ot_full[0:DH, :, ssl],
                    po[0:DH, :].rearrange("p (c s) -> p c s", c=NCB),
                    rr[0:DH, 0:512].rearrange("p (c s) -> p c s", c=NCB))
                nc.vector.tensor_mul(
                    ot_full[DH:P, :, ssl],
                    po[DH:P, :].rearrange("p (c s) -> p c s", c=NCB),
                    rr[DH:P, 512:1024].rearrange("p (c s) -> p c s", c=NCB))
            # O-projection outputs go through the attn pool's 2-bank tiles
            # (pairs of c_out blocks in the two bank halves) so the proj pool
            # frees up for the next group's Q/K/V immediately
            pods = []
            for cop in range(NCB // 2):
                pp2 = attn_ps.tile([P, 1024], f32, tag="attn", name="pp2")
                for half in range(2):
                    co = cop * 2 + half
                    dst = pp2[:, half * 512:(half + 1) * 512]
                    for ci in range(NCB):
                        nc.tensor.matmul(
                            dst,
                            lhsT=wo[ci][:, co * P:(co + 1) * P],
                            rhs=ot_full[:, ci, :],
                            start=(ci == 0), stop=(ci == NCB - 1))
                    pods.append(dst)
            return pods

        def height_pass():
            """Pass 1: height attention (seq along h, one per w).  Writes oh
            to the blocked scratch ohT2[hb][c, w, hi]."""
            wq, wk, wv, wo = (w_sb["wq_h"], w_sb["wk_h"], w_sb["wv_h"], w_sb["wo_h"])
            with ExitStack() as ctx:
                src_pool = ctx.enter_context(tc.tile_pool(name="src1", bufs=2))
                stage_pool = ctx.enter_context(tc.tile_pool(name="stg1", bufs=2))
                qk_pool = ctx.enter_context(tc.tile_pool(name="qk1", bufs=QK_BUFS))
                vt_pool = ctx.enter_context(tc.tile_pool(name="vt1", bufs=VT_BUFS))
                ot_pool = ctx.enter_context(tc.tile_pool(name="ot1", bufs=OT_BUFS))
                et_pool = ctx.enter_context(tc.tile_pool(name="et1", bufs=ET_BUFS))
                rr_pool = ctx.enter_context(tc.tile_pool(name="rr1", bufs=RR_BUFS))
                proj_ps = ctx.enter_context(tc.tile_pool(name="pps1", bufs=PROJ_BUFS, space="PSUM"))
                attn_ps = ctx.enter_context(tc.tile_pool(name="aps1", bufs=ATTN_BUFS, space="PSUM"))
                po_ps = ctx.enter_context(tc.tile_pool(name="pops1", bufs=PO_BUFS, space="PSUM"))
                pools = (qk_pool, vt_pool, ot_pool, et_pool, rr_pool, proj_ps, attn_ps, po_ps)

                for chunk in range(S // HC1):
                    q0 = chunk * HC1
                    src_t = src_pool.tile([P, NCB, HC1, S], f8, tag="src", name="src")
                    nc.sync.dma_start(out=src_t, in_=xq8[:, :, q0:q0 + HC1, :])
                    stage_t = []
                    for cb in range(NCB):
                        # stage layout (hb, w, hi): contiguous runs on both
                        # DMA sides of the blocked write
                        st = stage_pool.tile([P, HB, HC1, HC2], f32, tag=f"stg{cb}", name=f"stg{cb}")
                        stage_t.append(st)
                    for g in range(HC1 // G):
                        s0 = g * G
                        gsl = slice(s0, s0 + G)
                        pods = attn_group(src_t, gsl, s0, wq, wk, wv, wo, pools)
                        for co in range(NCB):
                            # pods: (p, 4 w-seq, 128 h) -> stage (hb, w in gsl, hi)
                            stage_op = nc.scalar.copy if STG1_ON_ACT else nc.vector.tensor_copy
                            stage_op(
                                stage_t[co][:, :, gsl, :].rearrange("p b q i -> p q b i"),
                                pods[co].rearrange("p (q b i) -> p q b i", q=G, b=HB))
                    for cb in range(NCB):
                        cs = slice(cb * P, (cb + 1) * P)
                        nc.sync.dma_start(
                            out=ohT2[:, cs, q0:q0 + HC1, :].rearrange("b c w i -> c b w i"),
                            in_=stage_t[cb])

        def width_pass():
            """Pass 2: width attention (seq along w, one per h).  h-chunk =
            hb block; out = xs + oh + ow in natural layout."""
            wq, wk, wv, wo = (w_sb["wq_w"], w_sb["wk_w"], w_sb["wv_w"], w_sb["wo_w"])
            with ExitStack() as ctx:
                src_pool = ctx.enter_context(tc.tile_pool(name="src2", bufs=2))
                resid_pool = ctx.enter_context(tc.tile_pool(name="res2", bufs=2))
                oh_pool = ctx.enter_context(tc.tile_pool(name="oh2", bufs=2))
                stage_pool = ctx.enter_context(tc.tile_pool(name="stg2", bufs=2))
                qk_pool = ctx.enter_context(tc.tile_pool(name="qk2", bufs=QK_BUFS))
                vt_pool = ctx.enter_context(tc.tile_pool(name="vt2", bufs=VT_BUFS))
                ot_pool = ctx.enter_context(tc.tile_pool(name="ot2", bufs=OT_BUFS))
                et_pool = ctx.enter_context(tc.tile_pool(name="et2", bufs=ET_BUFS))
                rr_pool = ctx.enter_context(tc.tile_pool(name="rr2", bufs=RR_BUFS))
                proj_ps = ctx.enter_context(tc.tile_pool(name="pps2", bufs=PROJ_BUFS, space="PSUM"))
                attn_ps = ctx.enter_context(tc.tile_pool(name="aps2", bufs=ATTN_BUFS, space="PSUM"))
                po_ps = ctx.enter_context(tc.tile_pool(name="pops2", bufs=PO_BUFS, space="PSUM"))
                pools = (qk_pool, vt_pool, ot_pool, et_pool, rr_pool, proj_ps, attn_ps, po_ps)

                for hb in range(HB):
                    q0 = hb * HC2
                    src_t = src_pool.tile([P, NCB, HC2, S], f8, tag="src", name="src")
                    nc.sync.dma_start(out=src_t, in_=xw8[:, :, q0:q0 + HC2, :])
                    resid_t, stage_t = [], []
                    for cb in range(NCB):
                        cs = slice(cb * P, (cb + 1) * P)
                        rt = resid_pool.tile([P, HC2, S], f32, tag=f"res{cb}", name=f"res{cb}")
                        nc.sync.dma_start(out=rt, in_=xf[cs, q0:q0 + HC2, :])
                        resid_t.append(rt)
                        oht = oh_pool.tile([P, S, HC2], f32, tag=f"oh{cb}", name=f"oh{cb}")
                        nc.sync.dma_start(out=oht, in_=ohT2[hb, cs, :, :])
                        # fold oh into the residual once per chunk
                        nc.gpsimd.tensor_tensor(
                            out=rt, in0=rt,
                            in1=oht.rearrange("p w i -> p i w"),
                            op=mybir.AluOpType.add)
                        st = stage_pool.tile([P, HC2, S], f32, tag=f"stg{cb}", name=f"stg{cb}")
                        stage_t.append(st)
                    for g in range(HC2 // G):
                        s0 = g * G
                        gsl = slice(s0, s0 + G)
                        pods = attn_group(src_t, gsl, s0, wq, wk, wv, wo, pools)
                        for co in range(NCB):
                            nc.vector.tensor_add(
                                stage_t[co][:, gsl, :],
                                pods[co].rearrange("p (q s) -> p q s", q=G),
                                resid_t[co][:, gsl, :])
                    for cb in range(NCB):
                        cs = slice(cb * P, (cb + 1) * P)
                        nc.sync.dma_start(out=out[cs, q0:q0 + HC2, :], in_=stage_t[cb])

        if timed:
            with tc.For_i(0, reps):
                height_pass()
                width_pass()
        else:
            height_pass()
            width_pass()

    nc.compile()
    return nc


def _get_program():
    global _PROG
    if _PROG is None:
        _PROG = _build_program()
    return _PROG


def kernel(xs, Wq_h, Wk_h, Wv_h, Wo_h, Wq_w, Wk_w, Wv_w, Wo_w):
    from concourse.bass_utils import run_bass_kernel_spmd

    nc = _get_program()

    w8map = {
        "wq_w": Wq_w, "wk_w": Wk_w, "wv_w": Wv_w,
        "wq_h": Wq_h, "wk_h": Wk_h, "wv_h": Wv_h,
    }
    w8_np = {}
    for n, w in w8map.items():
        wt = np.asarray(w, dtype=np.float32).T * WS          # (c_in, c_out)
        w8 = np.clip(wt, -240, 240).astype(_E4M3)
        w8_np[n] = np.ascontiguousarray(
            w8.reshape(NCB, P, C).transpose(1, 0, 2))        # (p, ci, co)
    wo_np = {
        "wo_w": np.ascontiguousarray(np.asarray(Wo_w, dtype=np.float32).T / WS).astype(_BF16),
        "wo_h": np.ascontiguousarray(np.asarray(Wo_h, dtype=np.float32).T / WS).astype(_BF16),
    }

    xs = np.asarray(xs, dtype=np.float32)
    in_maps = []
    for b in range(NCORES):
        xb = np.ascontiguousarray(xs[b])                        # (C, H, W) f32
        x8n = np.clip(xb, -240, 240).astype(_E4M3)
        xw8 = np.ascontiguousarray(
            x8n.reshape(NCB, P, S, S).transpose(1, 0, 2, 3))    # (p, ci, h, w)
        x8s = np.ascontiguousarray(np.swapaxes(x8n, 1, 2))      # (C, W, H)
        xq8 = np.ascontiguousarray(
            x8s.reshape(NCB, P, S, S).transpose(1, 0, 2, 3))    # (p, ci, w, h)
        in_maps.append({"xf": xb, "xq8": xq8, "xw8": xw8, **w8_np, **wo_np})

    res = run_bass_kernel_spmd(nc, in_maps, core_ids=list(range(NCORES)))
    return np.stack([res.results[b]["out"] for b in range(NCORES)], axis=0)



# revision 38
# speedup vs baseline: 2.3135x; 1.2268x over previous
"""Axial attention block (B=8, C=512, H=W=128, 8 heads) on 8 Trainium2 cores.

Sharding: data-parallel over batch — one batch element per NeuronCore. Each
core runs both axial passes on its (C, H, W) slice and produces the full
residual sum xs + oh + ow.

Q/K/V projections run in fp8(e4m3) DoubleRow mode (2 k-tiles per matmul —
HW-microbenched at 2.17x over bf16 at N=512): x is quantized to e4m3 on the
host in both layouts with a k-subtile axis ([p, ci, seq, s], channel =
ci*128 + p), and the Q/K/V weights are pre-scaled by WS=16 so their values
sit in e4m3's normal range.  The 16x comes out in the exp scale (scores
carry WS^2) and in Wo (pre-divided by WS on the host).  Scores/AV/
O-projection stay bf16; numpy-validated rel err ~1.6e-2 vs the 2e-2 gate.

Scheduling: engines consume their queues in order, so each pass emits a
software-pipelined instruction weave.  Per steady-state group i:
  - group i's per-seq chain (scores -> exp[ScalarE] -> denominator+AV ->
    reciprocal+normalize[VectorE]) is interleaved, matmul-burst by
    matmul-burst, with group i+1's Q/K/V DoubleRow projections so TensorE
    never drains while ScalarE/VectorE work through the chain;
  - PSUM evacuation copies are emitted right after the next chain step so
    engine-queue order can never deadlock against tile-pool buffer reuse
    (a copy gating a later pool reallocation always precedes, in its
    engine's queue, any exp that later chain matmuls wait on);
  - group i-1's O-projection + staging runs after the weave, filling the
    chain's tail latency.
PSUM: proj pool (Q/K/V psums + softmax-denominator r2) 2x[P,512], score
pool (st2 + deferred O-proj psums) 2x[P,1024], AV pool 2x[P,512] = 8 banks.

Per-sequence attention (S=128, dh=64): scoresT = K^T.T @ Q^T per head in
(s_k, s_q) layout, parity-split over two PSUM banks; exp on ScalarE (max-
subtraction skipped — scaled scores are bounded ~7); denominators via M=64
ones-matmuls on TensorE into the row-half of a single r2 bank; compact
[P, 512] reciprocal so the normalize is one full-partition VectorE multiply
(cost is per-column); AV packs all 8 heads into one PSUM bank in (c, s_q)
layout; O-projection batched over 4 sequences.
"""
import os
import numpy as np
import ml_dtypes

P = 128          # partitions
C = 512          # channels
S = 128          # sequence length (H and W)
NCB = C // P     # channel blocks
NH = 8           # heads
DH = C // NH     # head dim
G = 4            # sequences per projection group
HC1 = 16         # w-chunk, height pass
HC2 = 8          # h-chunk, width pass (= hi block size of ohT2)
HB = S // HC2    # number of h blocks
NCORES = 8
WS = 16.0        # fp8 weight pre-scale (Q/K/V)

_BF16 = ml_dtypes.bfloat16
_E4M3 = ml_dtypes.float8_e4m3

# schedule-tuning knobs (env-overridable for experiments)
PROJ_BUFS = int(os.environ.get("K_PROJ_BUFS", "2"))
ATTN_BUFS = int(os.environ.get("K_ATTN_BUFS", "2"))
ET_BUFS = int(os.environ.get("K_ET_BUFS", "2"))
QK_BUFS = int(os.environ.get("K_QK_BUFS", "2"))
VT_BUFS = int(os.environ.get("K_VT_BUFS", "2"))
OT_BUFS = int(os.environ.get("K_OT_BUFS", "2"))
RR_BUFS = int(os.environ.get("K_RR_BUFS", "2"))
PO_BUFS = int(os.environ.get("K_PO_BUFS", "2"))
QT1_ENG = os.environ.get("K_QT1_ENG", "dve")   # pass-1 qt copies
EXP_SPLIT = int(os.environ.get("K_EXP_SPLIT", "0"))
PSUM_SHARED = int(os.environ.get("K_PSUM_SHARED", "1"))

_PROG = None  # cached compiled Bass program


def _build_program(reps=None):
    """Build the Bass program.  reps=None: normal external-I/O program.
    reps=R: timing variant — body wrapped in a hardware For_i loop run R
    times, inputs Internal (no host transfer)."""
    from contextlib import ExitStack
    import concourse.tile as tile
    from concourse import bacc, mybir

    f32 = mybir.dt.float32
    bf = mybir.dt.bfloat16
    f8 = mybir.dt.float8e4
    Exp = mybir.ActivationFunctionType.Exp
    DR = mybir.MatmulPerfMode.DoubleRow

    nc = bacc.Bacc("TRN2", target_bir_lowering=False, debug=False)

    timed = reps is not None
    kio = {} if timed else {"kind": "ExternalInput"}
    xf = nc.dram_tensor("xf", [C, S, S], f32, **kio).ap()
    xq8 = nc.dram_tensor("xq8", [P, NCB, S, S], f8, **kio).ap()   # (p, ci, w, h)
    xw8 = nc.dram_tensor("xw8", [P, NCB, S, S], f8, **kio).ap()   # (p, ci, h, w)
    w8names = ["wq_w", "wk_w", "wv_w", "wq_h", "wk_h", "wv_h"]
    w8t = {n: nc.dram_tensor(n, [P, NCB, C], f8, **kio).ap() for n in w8names}
    wonames = ["wo_w", "wo_h"]
    wot = {n: nc.dram_tensor(n, [C, C], bf, **kio).ap() for n in wonames}
    ohT2 = nc.dram_tensor("ohT2", [HB, C, S, HC2], f32).ap()
    out = nc.dram_tensor("out", [C, S, S], f32, kind="ExternalOutput").ap()

    with tile.TileContext(nc) as tc, ExitStack() as topctx:
        const = topctx.enter_context(tc.tile_pool(name="const", bufs=1))

        w_sb = {}
        for n in w8names:
            t = const.tile([P, NCB, C], f8, tag=f"w_{n}", name=f"w_{n}")
            nc.sync.dma_start(out=t, in_=w8t[n])
            w_sb[n] = t
        for n in wonames:
            tiles = []
            for ci in range(NCB):
                t = const.tile([P, C], bf, tag=f"w_{n}_{ci}", name=f"w_{n}_{ci}")
                nc.sync.dma_start(out=t, in_=wot[n][ci * P:(ci + 1) * P, :])
                tiles.append(t)
            w_sb[n] = tiles
        ones_sb = const.tile([P, P], bf, tag="ones", name="ones")
        nc.vector.memset(ones_sb, 1.0)

        def copy_on(eng, dst, src):
            if eng == "dve":
                nc.vector.tensor_copy(dst, src)
            else:
                nc.scalar.copy(dst, src)

        def run_pass(is_h):
            """Emit one axial pass as a woven instruction stream."""
            if is_h:
                wq, wk, wv = w_sb["wq_h"], w_sb["wk_h"], w_sb["wv_h"]
                wo = w_sb["wo_h"]
                HC, x8 = HC1, xq8
                qt_eng = QT1_ENG
            else:
                wq, wk, wv = w_sb["wq_w"], w_sb["wk_w"], w_sb["wv_w"]
                wo = w_sb["wo_w"]
                HC, x8 = HC2, xw8
                qt_eng = "act"
            nchunks = S // HC
            gpc = HC // G
            ngroups = nchunks * gpc

            with ExitStack() as ctx:
                src_pool = ctx.enter_context(tc.tile_pool(name="src", bufs=2))
                stage_pool = ctx.enter_context(tc.tile_pool(name="stg", bufs=2))
                qk_pool = ctx.enter_context(tc.tile_pool(name="qk", bufs=QK_BUFS))
                vt_pool = ctx.enter_context(tc.tile_pool(name="vt", bufs=VT_BUFS))
                ot_pool = ctx.enter_context(tc.tile_pool(name="ot", bufs=OT_BUFS))
                et_pool = ctx.enter_context(tc.tile_pool(name="et", bufs=ET_BUFS))
                rr_pool = ctx.enter_context(tc.tile_pool(name="rr", bufs=RR_BUFS))
                if PSUM_SHARED:
                    proj_ps = ctx.enter_context(tc.tile_pool(name="pps", bufs=4, space="PSUM"))
                    attn_ps = proj_ps
                else:
                    proj_ps = ctx.enter_context(tc.tile_pool(name="pps", bufs=PROJ_BUFS, space="PSUM"))
                    attn_ps = ctx.enter_context(tc.tile_pool(name="aps", bufs=ATTN_BUFS, space="PSUM"))
                if not is_h:
                    resid_pool = ctx.enter_context(tc.tile_pool(name="res", bufs=2))
                    oh_pool = ctx.enter_context(tc.tile_pool(name="oh", bufs=2))

                chunks = {}   # chunk index -> dict of per-chunk tiles
                gstate = {}   # group index -> dict of per-group tiles

                def ensure_chunk(c):
                    if c in chunks or c >= nchunks:
                        return
                    q0 = c * HC
                    t = src_pool.tile([P, NCB, HC, S], f8, tag="src", name="src")
                    nc.sync.dma_start(out=t, in_=x8[:, :, q0:q0 + HC, :])
                    d = {"src": t, "q0": q0}
                    if is_h:
                        d["stage"] = [
                            stage_pool.tile([P, HB, HC, HC2], f32,
                                            tag=f"stg{cb}", name=f"stg{cb}")
                            for cb in range(NCB)]
                    else:
                        d["stage"] = [
                            stage_pool.tile([P, HC, S], f32,
                                            tag=f"stg{cb}", name=f"stg{cb}")
                            for cb in range(NCB)]
                        d["resid"], d["fold"] = [], []
                        for cb in range(NCB):
                            cs = slice(cb * P, (cb + 1) * P)
                            rt = resid_pool.tile([P, HC, S], f32,
                                                 tag=f"res{cb}", name=f"res{cb}")
                            nc.sync.dma_start(out=rt, in_=xf[cs, q0:q0 + HC, :])
                            d["resid"].append(rt)
                            oht = oh_pool.tile([P, S, HC2], f32,
                                               tag=f"oh{cb}", name=f"oh{cb}")
                            nc.sync.dma_start(out=oht, in_=ohT2[c, cs, :, :])
                            d["fold"].append((rt, oht))
                    chunks[c] = d

                def fold_chunk(c):
                    # pass-2: oh folds into the residual on GpSimd
                    # (SBUF-to-SBUF; GpSimd cannot touch PSUM)
                    for rt, oht in chunks[c].pop("fold", []):
                        nc.gpsimd.tensor_tensor(
                            out=rt, in0=rt,
                            in1=oht.rearrange("p w i -> p i w"),
                            op=mybir.AluOpType.add)

                def g_chunk(i):
                    return i // gpc

                def g_state(i):
                    if i not in gstate:
                        gstate[i] = {"qt": {}, "kt": {}, "vt": {}, "pp": {},
                                     "pv": {}, "et": {}, "ot": None}
                    return gstate[i]

                # --- projection units (group i): [P, 1024] 2-bank psum pairs
                # so each unit is 4 DoubleRow matmuls + ONE wide evacuation
                # copy (halves the per-instruction engine overhead).
                # u: 0 = q co01, 1 = k co01, 2 = q co23, 3 = k co23;
                # v units: 0 = sq01, 1 = sq23.
                def proj_mm(i, u, half):
                    stq = g_state(i)
                    wmat = wq if u % 2 == 0 else wk
                    cop = u // 2
                    s0 = (i % gpc) * G
                    gsl = slice(s0, s0 + G)
                    x8_t = chunks[g_chunk(i)]["src"]
                    if half == 0:
                        stq["pp"][u] = proj_ps.tile([P, 2 * G * S], f32,
                                                    tag="proj", name="pp")
                    pp = stq["pp"][u]
                    co = cop * 2 + half
                    for k2 in range(NCB // 2):
                        ks = slice(2 * k2, 2 * k2 + 2)
                        nc.tensor.matmul(
                            pp[:, half * 512:(half + 1) * 512],
                            lhsT=wmat[:, ks, co * P:(co + 1) * P],
                            rhs=x8_t[:, ks, gsl, :],
                            start=(k2 == 0), stop=(k2 == NCB // 2 - 1),
                            perf_mode=DR)

                def proj_copy(i, u):
                    stq = g_state(i)
                    nm = "qt" if u % 2 == 0 else "kt"
                    cop = u // 2
                    sb_t = qk_pool.tile([P, 2 * G * S], bf, tag=f"{nm}{cop}",
                                        name=f"{nm}{cop}")
                    copy_on(qt_eng if nm == "qt" else "act", sb_t, stq["pp"].pop(u))
                    stq[nm][cop] = sb_t

                def v_mm(i, vp, half):
                    stq = g_state(i)
                    s0 = (i % gpc) * G
                    x8_t = chunks[g_chunk(i)]["src"]
                    if half == 0:
                        stq["pv"][vp] = proj_ps.tile([P, 2 * C], f32,
                                                     tag="proj", name="pv")
                    pv = stq["pv"][vp]
                    sq = vp * 2 + half
                    for k2 in range(NCB // 2):
                        ks = slice(2 * k2, 2 * k2 + 2)
                        nc.tensor.matmul(
                            pv[:, half * C:(half + 1) * C],
                            lhsT=x8_t[:, ks, s0 + sq, :], rhs=wv[:, ks, :],
                            start=(k2 == 0), stop=(k2 == NCB // 2 - 1),
                            perf_mode=DR)

                def v_copy(i, vp):
                    stq = g_state(i)
                    vt = vt_pool.tile([P, 2 * C], bf, tag=f"vt{vp}", name=f"vt{vp}")
                    copy_on("act", vt, stq["pv"].pop(vp))
                    stq["vt"][vp] = vt

                # --- per-seq chain (group i) ---
                def sc(i, sq):
                    stq = g_state(i)
                    st2 = attn_ps.tile([P, 1024], f32, tag="proj" if PSUM_SHARED else "attn", name="st2")
                    for h in range(NH):
                        par, cb = h % 2, h // 2
                        rows = slice(par * DH, (par + 1) * DH)
                        co = h // 2
                        ssl = slice((co % 2) * 512 + sq * S,
                                    (co % 2) * 512 + (sq + 1) * S)
                        nc.tensor.matmul(
                            st2[:, par * 512 + cb * S:par * 512 + (cb + 1) * S],
                            lhsT=stq["kt"][co // 2][rows, ssl],
                            rhs=stq["qt"][co // 2][rows, ssl],
                            start=True, stop=True)
                    et = et_pool.tile([P, 1024], bf, tag="et", name="et")
                    esc = DH ** -0.5 / (WS * WS)
                    if EXP_SPLIT:
                        nc.scalar.activation(out=et[:, 0:512], in_=st2[:, 0:512],
                                             func=Exp, scale=esc)
                        nc.scalar.activation(out=et[:, 512:1024],
                                             in_=st2[:, 512:1024],
                                             func=Exp, scale=esc)
                    else:
                        nc.scalar.activation(out=et, in_=st2, func=Exp, scale=esc)
                    stq["et"][sq] = et

                def dav(i, sq):
                    stq = g_state(i)
                    if stq["ot"] is None:
                        stq["ot"] = ot_pool.tile([P, NCB, G * S], bf,
                                                 tag="ot", name="ot")
                    ot_full = stq["ot"]
                    et = stq["et"].pop(sq)
                    ssl = slice(sq * S, (sq + 1) * S)
                    # r2 (softmax denominators) in the first bank, po (AV
                    # accumulators) in the second bank of one pool tile
                    r2po = proj_ps.tile([P, 2 * 512], f32, tag="proj", name="r2po")
                    r2 = r2po[:, 0:512]
                    po = r2po[:, 512:1024]
                    rr = rr_pool.tile([P, 512], f32, tag="rr", name="rr")
                    vt = stq["vt"][sq // 2]
                    voff = (sq % 2) * C
                    for par in range(2):
                        half = slice(par * 512, (par + 1) * 512)
                        rows = slice(par * DH, (par + 1) * DH)
                        nc.tensor.matmul(r2[rows, :], lhsT=ones_sb[:, rows],
                                         rhs=et[:, half], start=True, stop=True)
                        for cb in range(NCB):
                            h = 2 * cb + par
                            nc.tensor.matmul(
                                po[par * DH:(par + 1) * DH, cb * S:(cb + 1) * S],
                                lhsT=vt[:, voff + h * DH:voff + (h + 1) * DH],
                                rhs=et[:, par * 512 + cb * S:par * 512 + (cb + 1) * S],
                                start=True, stop=True)
                    # single full-partition reciprocal: partition-sliced
                    # DVE-ucode ops NaN on HW (see trn2-hw-pitfalls)
                    nc.vector.reciprocal_approx_fast(out=rr, in_=r2)
                    nc.vector.tensor_mul(
                        ot_full[:, :, ssl],
                        po.rearrange("p (c s) -> p c s", c=NCB),
                        rr.rearrange("p (c s) -> p c s", c=NCB))

                def emit_b(i):
                    """O-projection + staging for group i (deferred)."""
                    stq = gstate[i]
                    ot_full = stq["ot"]
                    c = g_chunk(i)
                    s0 = (i % gpc) * G
                    gsl = slice(s0, s0 + G)
                    ch = chunks[c]
                    pods = []
                    for cop in range(NCB // 2):
                        pp2 = attn_ps.tile([P, 1024], f32, tag="proj" if PSUM_SHARED else "attn", name="pp2")
                        for half in range(2):
                            co = cop * 2 + half
                            dst = pp2[:, half * 512:(half + 1) * 512]
                            for ci in range(NCB):
                                nc.tensor.matmul(
                                    dst,
                                    lhsT=wo[ci][:, co * P:(co + 1) * P],
                                    rhs=ot_full[:, ci, :],
                                    start=(ci == 0), stop=(ci == NCB - 1))
                            pods.append(dst)
                    for co in range(NCB):
                        if is_h:
                            # pods: (p, 4 w, 128 h) -> stage (hb, w in gsl, hi)
                            nc.scalar.copy(
                                ch["stage"][co][:, :, gsl, :].rearrange("p b q i -> p q b i"),
                                pods[co].rearrange("p (q b i) -> p q b i", q=G, b=HB))
                        else:
                            nc.vector.tensor_add(
                                ch["stage"][co][:, gsl, :],
                                pods[co].rearrange("p (q s) -> p q s", q=G),
                                ch["resid"][co][:, gsl, :])
                    if i % gpc == gpc - 1:   # last group of its chunk
                        q0 = ch["q0"]
                        for cb in range(NCB):
                            cs = slice(cb * P, (cb + 1) * P)
                            if is_h:
                                nc.sync.dma_start(
                                    out=ohT2[:, cs, q0:q0 + HC, :].rearrange("b c w i -> c b w i"),
                                    in_=ch["stage"][cb])
                            else:
                                nc.sync.dma_start(out=out[cs, q0:q0 + HC, :],
                                                  in_=ch["stage"][cb])
                    del gstate[i]

                def emit_proj_full(i):
                    """Unwoven projections (prologue group)."""
                    for u in range(4):
                        proj_mm(i, u, 0)
                        proj_mm(i, u, 1)
                        proj_copy(i, u)
                    for vp in range(2):
                        v_mm(i, vp, 0)
                        v_mm(i, vp, 1)
                        v_copy(i, vp)

                def weave(i):
                    """Chain of group i, woven with projections of i+1."""
                    nxt = i + 1 if i + 1 < ngroups else None
                    if nxt is not None:
                        ensure_chunk(g_chunk(nxt))
                    # emission schedule: chain steps with 2-matmul fill bursts
                    # between; each [P,1024] psum pair gets ONE wide
                    # evacuation copy after both halves are in
                    sched = ["sc0", "sc1", "m00", "m01", "dav0", "c0x",
                             "m10", "sc2", "m11", "dav1", "c1x", "m20",
                             "sc3", "m21", "dav2", "c2x", "m30",
                             "dav3", "m31", "c3x",
                             "w00", "w01", "c4x", "w10", "w11", "c5x"]
                    for item in sched:
                        kind = item[0]
                        if kind == "s":
                            sc(i, int(item[2]))
                        elif kind == "d":
                            dav(i, int(item[3]))
                            if not is_h and item[3] == "0" and i % gpc == 0:
                                fold_chunk(g_chunk(i))
                        elif nxt is None:
                            continue
                        elif kind == "m":
                            proj_mm(nxt, int(item[1]), int(item[2]))
                        elif kind == "w":
                            v_mm(nxt, int(item[1]), int(item[2]))
                        elif kind == "c":
                            u = int(item[1])
                            if u < 4:
                                proj_copy(nxt, u)
                            else:
                                v_copy(nxt, u - 4)

                ensure_chunk(0)
                emit_proj_full(0)
                for i in range(ngroups):
                    weave(i)
                    if i > 0:
                        emit_b(i - 1)
                emit_b(ngroups - 1)

        if timed:
            with tc.For_i(0, reps):
                run_pass(True)
                run_pass(False)
        else:
            run_pass(True)
            run_pass(False)

    nc.compile()
    return nc


def _get_program():
    global _PROG
    if _PROG is None:
        _PROG = _build_program()
    return _PROG


def kernel(xs, Wq_h, Wk_h, Wv_h, Wo_h, Wq_w, Wk_w, Wv_w, Wo_w):
    from concourse.bass_utils import run_bass_kernel_spmd

    nc = _get_program()

    w8map = {
        "wq_w": Wq_w, "wk_w": Wk_w, "wv_w": Wv_w,
        "wq_h": Wq_h, "wk_h": Wk_h, "wv_h": Wv_h,
    }
    w8_np = {}
    for n, w in w8map.items():
        wt = np.asarray(w, dtype=np.float32).T * WS          # (c_in, c_out)
        w8 = np.clip(wt, -240, 240).astype(_E4M3)
        # (ci*P + p, co) -> (p, ci, co)
        w8_np[n] = np.ascontiguousarray(
            w8.reshape(NCB, P, C).transpose(1, 0, 2))
    wo_np = {
        "wo_w": np.ascontiguousarray(np.asarray(Wo_w, dtype=np.float32).T / WS).astype(_BF16),
        "wo_h": np.ascontiguousarray(np.asarray(Wo_h, dtype=np.float32).T / WS).astype(_BF16),
    }

    xs = np.asarray(xs, dtype=np.float32)
    in_maps = []
    for b in range(NCORES):
        xb = np.ascontiguousarray(xs[b])                        # (C, H, W) f32
        x8n = np.clip(xb, -240, 240).astype(_E4M3)              # (C, H, W)
        xw8 = np.ascontiguousarray(
            x8n.reshape(NCB, P, S, S).transpose(1, 0, 2, 3))    # (p, ci, h, w)
        x8s = np.ascontiguousarray(np.swapaxes(x8n, 1, 2))      # (C, W, H)
        xq8 = np.ascontiguousarray(
            x8s.reshape(NCB, P, S, S).transpose(1, 0, 2, 3))    # (p, ci, w, h)
        in_maps.append({"xf": xb, "xq8": xq8, "xw8": xw8, **w8_np, **wo_np})

    res = run_bass_kernel_spmd(nc, in_maps, core_ids=list(range(NCORES)))
    return np.stack([res.results[b]["out"] for b in range(NCORES)], axis=0)
